# revision 52
# baseline (speedup 1.0000x reference)
"""DSTAGNN attention submodule Trainium2 kernel.

Computes, per (b, f):
    q = x @ W_Q ; k = x @ W_K ; v = x @ W_V          (multi-head, H=8, dk=32)
    scores = q @ k^T / sqrt(dk) + res_att            (returned output #2)
    attn = softmax(scores, axis=QUERY)               (softmax over s, not t!)
    ctx = attn @ v ; out = LN(ctx @ W_fc + x)        (returned output #1)

Sharding: data-parallel over B=8 across the 8 NeuronCores. Each core runs an
identical Bass program over its [F=4, S=512, D=256] slice; host gathers.
"""

import os
import sys
from contextlib import ExitStack

for _p in ("/opt/trn_rl_repo", os.path.expanduser("~/.axon_site/_ro/trn_rl_repo")):
    if os.path.isdir(_p) and _p not in sys.path:
        sys.path.insert(0, _p)

import numpy as np

import concourse.bass as bass
import concourse.tile as tile
from concourse import mybir
from concourse.masks import make_identity

F32 = mybir.dt.float32
F32R = mybir.dt.float32r
AF = mybir.ActivationFunctionType
P = 128

B, F, S, D = 8, 4, 512, 256
H, DK, DV = 8, 32, 32
LN_EPS = 1e-5

ST = S // P          # 4 sequence tiles of 128
KD = D // P          # 2 contraction tiles over D
OT = (H * DK) // P   # 2 tiles over the packed head dim
HPT = 3              # heads per 128-partition tile (matmul operand base
                     # partition must be in {0, 32, 64}, so 3 x 32 rows)
NG = 3               # number of head groups (3 + 3 + 2 heads)
GRP = [(0, 96), (96, 96), (192, 64)]  # (hv col offset, width) per group

# tuning knobs (read at build_program time)
TUNE = dict(res_bufs=4, sc_bufs=7, ps_proj=2, ps_qk=2, ps_ts=2, ps_ctx=2,
            ET_bufs=2, batch2=False, store_ring_act=False, pipe_at=2, defer_st=7, ln_pool=False,
            split_dma=False)



def _emit(ctx: ExitStack, tc: tile.TileContext, nF: int, nH: int, tensors):
    nc = tc.nc
    x_d, res_d, wq_d, wk_d, wv_d, wfc_d, g_d, b_d, scores_d, out_d = tensors

    singles = ctx.enter_context(tc.tile_pool(name="singles", bufs=1))
    perf = ctx.enter_context(tc.tile_pool(name="perf", bufs=2))
    perfh = ctx.enter_context(tc.tile_pool(name="perfh", bufs=TUNE["ET_bufs"]))
    respool = ctx.enter_context(tc.tile_pool(name="respool", bufs=TUNE["res_bufs"]))
    ctxtp = ctx.enter_context(tc.tile_pool(name="ctxtp", bufs=1))
    stg = ctx.enter_context(tc.tile_pool(name="stg", bufs=1))
    scpool = ctx.enter_context(tc.tile_pool(name="scpool", bufs=TUNE["sc_bufs"]))
    ypool = ctx.enter_context(tc.tile_pool(name="ypool", bufs=2))
    # PSUM budget is 8 banks total
    assert TUNE["ps_proj"] + TUNE["ps_qk"] + TUNE["ps_ts"] + TUNE["ps_ctx"] <= 8
    ps_proj = ctx.enter_context(tc.tile_pool(name="ps_proj", bufs=TUNE["ps_proj"], space="PSUM"))
    ps_qk = ctx.enter_context(tc.tile_pool(name="ps_qk", bufs=TUNE["ps_qk"], space="PSUM"))
    ps_ts = ctx.enter_context(tc.tile_pool(name="ps_ts", bufs=TUNE["ps_ts"], space="PSUM"))
    ps_ctx = ctx.enter_context(tc.tile_pool(name="ps_ctx", bufs=TUNE["ps_ctx"], space="PSUM"))

    ident = singles.tile([P, P], F32)
    make_identity(nc, ident)
    ident_r = singles.tile([P, P], F32R)
    nc.scalar.copy(out=ident_r, in_=ident)

    def load_w(dram, nkt, nm):
        w32 = stg.tile([P, nkt, H * DK], F32, name=nm + "32", tag="wstg")
        nc.sync.dma_start(out=w32, in_=dram.rearrange("(kt p) n -> p kt n", p=P))
        # matmul inputs must be fp32r-rounded; ScalarE rounds on write
        w = singles.tile([P, nkt, H * DK], F32R, name=nm, tag=nm)
        nc.scalar.copy(out=w, in_=w32)
        return w

    wq_sb = load_w(wq_d, KD, "wq_sb")
    wk_sb = load_w(wk_d, KD, "wk_sb")
    wv_sb = load_w(wv_d, KD, "wv_sb")
    wfc32 = stg.tile([DV, H, D], F32, name="wfc32", tag="wstg")
    nc.sync.dma_start(out=wfc32, in_=wfc_d.rearrange("p (h n) -> p h n", h=H))
    wfc_sb = singles.tile([DV, H, D], F32R, name="wfc_sb", tag="wfc_sb")
    nc.scalar.copy(out=wfc_sb, in_=wfc32)

    # ln affine params broadcast across all 128 partitions
    def bcast_vec(dram, nm):
        t = singles.tile([P, D], F32, name=nm, tag=nm)
        src = bass.AP(tensor=dram.tensor, offset=dram.offset,
                      ap=[[0, P]] + list(dram.ap))
        nc.sync.dma_start(out=t, in_=src)
        return t

    g_sb = bcast_vec(g_d[:], "g_sb")
    b_sb = bcast_vec(b_d[:], "b_sb")
    eps_sb = singles.tile([P, 1], F32)
    nc.vector.memset(eps_sb, LN_EPS)
    # dummy Ln pins the natural_log_exp table set (holds Ln AND Exp) so the
    # per-f layernorm below never forces an ACT table switch
    pin_sb = singles.tile([P, 1], F32)
    nc.scalar.activation(out=pin_sb, in_=eps_sb, func=AF.Ln)
    mv_all = singles.tile([P, nF * ST, 2], F32)

    def emit_proj(f):
        # ---- load x, build xT via PE transpose ----
        x_sb = perf.tile([P, ST, D], F32, tag="x", name=f"x{f}")
        nc.sync.dma_start(out=x_sb, in_=x_d[f].rearrange("(o p) d -> p o d", p=P))
        xT_sb = perf.tile([P, KD, S], F32R, tag="xT", name=f"xT{f}")
        for dt_ in range(KD):
            pst = ps_proj.tile([P, S], F32, tag="proj", name=f"pstx{f}_{dt_}")
            for st in range(ST):
                nc.tensor.transpose(
                    pst[:, st * P:(st + 1) * P],
                    x_sb[:, st, dt_ * P:(dt_ + 1) * P],
                    ident,
                )
            nc.scalar.copy(out=xT_sb[:, dt_, :], in_=pst)

        # ---- projections: qT/kT in [o, s] layout (3 heads per 128-row
        # tile so head slices sit at legal base partitions); v natural ----
        qT_sb = perf.tile([P, NG, S], F32R, tag="qT", name=f"qT{f}")
        kT_sb = perf.tile([P, NG, S], F32R, tag="kT", name=f"kT{f}")
        for w_sb, dst in ((wq_sb, qT_sb), (wk_sb, kT_sb)):
            for g, (c0, gw) in enumerate(GRP):
                ps = ps_proj.tile([P, S], F32, tag="proj", name=f"psq{f}_{g}")
                for kt in range(KD):
                    nc.tensor.matmul(
                        ps[:gw], w_sb[:, kt, c0:c0 + gw],
                        xT_sb[:, kt, :],
                        start=(kt == 0), stop=(kt == KD - 1),
                    )
                nc.scalar.copy(out=dst[:gw, g, :], in_=ps[:gw])
        v_sb = perf.tile([P, ST, H * DV], F32, tag="v", name=f"v{f}")
        for g2 in range(ST // 2):
            ps = ps_proj.tile([P, 2, H * DV], F32, tag="proj", name=f"psv{f}_{g2}")
            for i2 in range(2):
                tt = g2 * 2 + i2
                for kt in range(KD):
                    nc.tensor.matmul(
                        ps[:, i2, :], xT_sb[:, kt, tt * P:(tt + 1) * P],
                        wv_sb[:, kt, :],
                        start=(kt == 0), stop=(kt == KD - 1),
                    )
            nc.scalar.copy(out=v_sb[:, g2 * 2:g2 * 2 + 2, :], in_=ps)
        return x_sb, qT_sb, kT_sb, v_sb

    cur = emit_proj(0)
    deferred = []  # (sc_sb, f, h2) score stores pushed past the compute drain
    for f in range(nF):
        x_sb, qT_sb, kT_sb, v_sb = cur
        nxt = None
        PIPE_AT = TUNE["pipe_at"]

        # ---- attention per head ----
        # per-head ctx^T rows live at partitions 0..32 of one packed tile
        # (4-byte matmuls require destination partition base 0)
        ctxT_sb = ctxtp.tile([DV, H, S], F32R, tag="ctxT")
        HB = 2 if TUNE["batch2"] else 1
        assert nH % HB == 0
        for h2 in range(nH // HB):
            # res load / scores store batched as one DMA per HB heads
            res_sb = respool.tile([P, HB, ST, S], F32, tag="res")
            res_src = res_d[f, HB * h2:HB * h2 + HB].rearrange(
                "i (o p) t -> p i o t", p=P)
            if TUNE["split_dma"]:
                hs = ST // 2
                nc.sync.dma_start(out=res_sb[:, :, :hs, :], in_=res_src[:, :, :hs, :])
                nc.sync.dma_start(out=res_sb[:, :, hs:, :], in_=res_src[:, :, hs:, :])
            else:
                nc.sync.dma_start(out=res_sb, in_=res_src)
            sc_sb = scpool.tile([P, HB, ST, S], F32R, tag="sc")
            for i in range(HB):
                h = HB * h2 + i
                ot_h, hp = h // HPT, (h % HPT) * DK  # group, base partition
                for st in range(ST):
                    ps = ps_qk.tile([P, S], F32, tag="qk")
                    nc.tensor.matmul(
                        ps,
                        qT_sb[hp:hp + DK, ot_h, st * P:(st + 1) * P],
                        kT_sb[hp:hp + DK, ot_h, :],
                        start=True, stop=True,
                    )
                    # fused: scores = qk + res, PSUM -> SBUF
                    i_add = nc.vector.tensor_add(
                        out=sc_sb[:, i, st, :], in0=ps, in1=res_sb[:, i, st, :])
                    if TUNE.get("tail_prio") and f == nF - 1 and h >= nH - 2:
                        i_add.bass_priority = -100
            if f == nF - 1 and h2 >= nH // HB - TUNE["defer_st"]:
                # defer the final stores so DMA stays busy through the
                # end-of-kernel compute drain
                deferred.append((sc_sb, f, h2))
            else:
                st_eng = nc.scalar if TUNE["store_ring_act"] else nc.sync
                st_eng.dma_start(  # stores can ride the ACT HWDGE ring
                    out=scores_d[f, HB * h2:HB * h2 + HB].rearrange(
                        "i (o p) t -> p i o t", p=P),
                    in_=sc_sb.bitcast(F32))
            for i in range(HB):
                h = HB * h2 + i
                # transposed scores -> exp + per-t row sums (softmax over s)
                ET_sb = perfh.tile([P, ST, S], F32R, tag="ET")
                cs_sb = perfh.tile([P, ST], F32, tag="cs")
                for tt in range(ST):
                    ps_t = ps_ts.tile([P, S], F32, tag="ts")
                    for st in range(ST):
                        nc.tensor.transpose(
                            ps_t[:, st * P:(st + 1) * P].bitcast(F32R),
                            sc_sb[:, i, st, tt * P:(tt + 1) * P],
                            ident_r,
                        )
                    nc.scalar.activation(
                        out=ET_sb[:, tt, :], in_=ps_t, func=AF.Exp,
                        accum_out=cs_sb[:, tt:tt + 1])
                rc_sb = perfh.tile([P, ST], F32, tag="rc")
                nc.vector.reciprocal(out=rc_sb, in_=cs_sb)
                vp_sb = perfh.tile([P, ST, DV], F32R, tag="vp")
                for tt in range(ST):
                    nc.vector.tensor_scalar_mul(
                        out=vp_sb[:, tt, :],
                        in0=v_sb[:, tt, h * DV:(h + 1) * DV],
                        scalar1=rc_sb[:, tt:tt + 1])
                ctx_ps = ps_ctx.tile([P, S], F32, tag="ctx")
                for tt in range(ST):
                    nc.tensor.matmul(
                        ctx_ps[:DV, :],
                        vp_sb[:, tt, :], ET_sb[:, tt, :],
                        start=(tt == 0), stop=(tt == ST - 1),
                    )
                nc.vector.tensor_copy(out=ctxT_sb[:, h, :], in_=ctx_ps[:DV, :])
                if h == PIPE_AT and f + 1 < nF:
                    # software-pipeline: emit next frame's projections here so
                    # the scheduler overlaps them with this frame's tail heads
                    nxt = emit_proj(f + 1)

        if nxt is None and f + 1 < nF:
            nxt = emit_proj(f + 1)
        cur = nxt

        # flush deferred stores ahead of the LN/out chain so the SP DMA ring
        # isn't head-of-line blocked behind the out store's LN dependency
        for sc_dv, fv, h2v in deferred:
            nc.sync.dma_start(
                out=scores_d[fv, HB * h2v:HB * h2v + HB].rearrange(
                    "i (o p) t -> p i o t", p=P),
                in_=sc_dv.bitcast(F32))
        deferred.clear()

        # ---- fc + residual + LN stats ----
        y_sb = ypool.tile([P, ST, D], F32, tag="y")
        stats_sb = perf.tile([P, ST, 6], F32, tag="stats")
        for g2 in range(ST // 2):
            ps = ps_proj.tile([P, 2, D], F32, tag="proj")
            for i2 in range(2):
                st = g2 * 2 + i2
                for hh in range(nH):
                    nc.tensor.matmul(
                        ps[:, i2, :],
                        ctxT_sb[:, hh, st * P:(st + 1) * P],
                        wfc_sb[:, hh, :],
                        start=(hh == 0), stop=(hh == nH - 1),
                    )
            for i2 in range(2):
                st = g2 * 2 + i2
                sl = f * ST + st
                nc.vector.tensor_add(
                    out=y_sb[:, st, :], in0=ps[:, i2, :], in1=x_sb[:, st, :])
                nc.vector.bn_stats(out=stats_sb[:, st, :], in_=y_sb[:, st, :])
                nc.vector.bn_aggr(out=mv_all[:, sl, :], in_=stats_sb[:, st, :])
                # per-s-tile layernorm + store keeps the kernel tail short:
                # rstd = exp(-0.5 * ln(var + eps)), same ACT table set as Exp
                lnv = perfh.tile([P, 1], F32, tag="lnv")
                nc.scalar.activation(out=lnv, in_=mv_all[:, sl, 1:2],
                                     func=AF.Ln, bias=eps_sb, scale=1.0)
                rstd = perfh.tile([P, 1], F32, tag="rstd")
                nc.scalar.activation(out=rstd, in_=lnv, func=AF.Exp, scale=-0.5)
                nc.vector.tensor_scalar(
                    out=y_sb[:, st, :], in0=y_sb[:, st, :],
                    scalar1=mv_all[:, sl, 0:1], scalar2=rstd,
                    op0=mybir.AluOpType.subtract, op1=mybir.AluOpType.mult)
                ln_eng = nc.gpsimd if TUNE["ln_pool"] else nc.vector
                ln_eng.tensor_mul(
                    out=y_sb[:, st, :], in0=y_sb[:, st, :], in1=g_sb)
                ln_eng.tensor_add(
                    out=y_sb[:, st, :], in0=y_sb[:, st, :], in1=b_sb)
                nc.sync.dma_start(
                    out=out_d[f].rearrange("(o p) d -> p o d", p=P)[:, st, :],
                    in_=y_sb[:, st, :])


def _split_matmul_waits(nc: bass.Bass) -> None:
    """Hoist extra sem waits onto same-engine EventSemaphore instructions.

    walrus codegen allows only one sync wait per instruction struct (two for
    EventSemaphore); Tile can attach several. Same-engine instructions
    execute in order, so moving all but one wait onto preceding
    EventSemaphores is semantics-preserving.
    """
    for fn in nc.m.functions:
        for blk in fn.blocks:
            insts = list(blk.instructions)
            out = []
            for inst in insts:
                si = inst.sync_info
                splittable = not isinstance(inst, mybir.InstEventSemaphore)
                if (splittable and si is not None
                        and len(si.on_wait) > 1):
                    extra = list(si.on_wait[:-1])
                    # EventSemaphore is the only struct that takes 2 waits;
                    # plain NOPs take none.
                    for j in range(0, len(extra), 2):
                        ev = mybir.InstEventSemaphore(
                            name=nc.get_next_instruction_name(),
                            sync_info=mybir.SyncInfo(
                                on_wait=extra[j:j + 2], on_update=[]),
                            bass_nofuse=True,
                            engine=inst.engine,
                        )
                        out.append(ev)
                    si.on_wait = [si.on_wait[-1]]
                    inst.sync_info = si
                out.append(inst)
            if len(out) != len(insts):
                del blk.instructions[:]
                for i in out:
                    blk.instructions.append(i)


def build_program(nF: int = F, nH: int = H, split_waits: bool = True) -> bass.Bass:
    nc = bass.Bass()
    x_d = nc.declare_dram_parameter("x", [nF, S, D], F32, isOutput=False)
    res_d = nc.declare_dram_parameter("res", [nF, nH, S, S], F32, isOutput=False)
    wq_d = nc.declare_dram_parameter("wq", [D, H * DK], F32, isOutput=False)
    wk_d = nc.declare_dram_parameter("wk", [D, H * DK], F32, isOutput=False)
    wv_d = nc.declare_dram_parameter("wv", [D, H * DV], F32, isOutput=False)
    wfc_d = nc.declare_dram_parameter("wfc", [DV, H * D], F32, isOutput=False)
    g_d = nc.declare_dram_parameter("g", [D], F32, isOutput=False)
    b_d = nc.declare_dram_parameter("b", [D], F32, isOutput=False)
    scores_d = nc.declare_dram_parameter("scores", [nF, nH, S, S], F32, isOutput=True)
    out_d = nc.declare_dram_parameter("out", [nF, S, D], F32, isOutput=True)
    tensors = (x_d, res_d, wq_d, wk_d, wv_d, wfc_d, g_d, b_d, scores_d, out_d)
    with tile.TileContext(nc) as tc:
        with ExitStack() as ctx:
            _emit(ctx, tc, nF, nH, tensors)
    if split_waits:  # CoreSim can't execute the inserted EventSemaphores
        _split_matmul_waits(nc)
    return nc


_CACHE: dict = {}


def _program() -> bass.Bass:
    if "nc" not in _CACHE:
        _CACHE["nc"] = build_program()
    return _CACHE["nc"]


def make_in_maps(x, res_att, W_Q, W_K, W_V, W_fc, ln_g, ln_b):
    f32 = lambda a: np.ascontiguousarray(np.asarray(a), dtype=np.float32)
    x, res_att = f32(x), f32(res_att)
    wq = f32(W_Q) / np.float32(np.sqrt(DK))  # fold 1/sqrt(dk) into W_Q
    wk, wv = f32(W_K), f32(W_V)
    # regroup W_fc rows per head: [H*DV, D] -> [DV, H*D]
    wfc = np.ascontiguousarray(
        f32(W_fc).reshape(H, DV, D).transpose(1, 0, 2).reshape(DV, H * D))
    g, b = f32(ln_g), f32(ln_b)
    return [
        dict(x=x[c], res=res_att[c], wq=wq, wk=wk, wv=wv, wfc=wfc, g=g, b=b)
        for c in range(B)
    ]


def kernel(x, res_att, W_Q, W_K, W_V, W_fc, ln_g, ln_b):
    from concourse.bass_utils import run_bass_kernel_spmd

    in_maps = make_in_maps(x, res_att, W_Q, W_K, W_V, W_fc, ln_g, ln_b)
    r = run_bass_kernel_spmd(_program(), in_maps, list(range(B)))
    out = np.stack([r.results[c]["out"] for c in range(B)])
    scores = np.stack([r.results[c]["scores"] for c in range(B)])
    return out, scores


# revision 55
# speedup vs baseline: 1.0061x; 1.0061x over previous
"""DSTAGNN attention submodule Trainium2 kernel.

Computes, per (b, f):
    q = x @ W_Q ; k = x @ W_K ; v = x @ W_V          (multi-head, H=8, dk=32)
    scores = q @ k^T / sqrt(dk) + res_att            (returned output #2)
    attn = softmax(scores, axis=QUERY)               (softmax over s, not t!)
    ctx = attn @ v ; out = LN(ctx @ W_fc + x)        (returned output #1)

Sharding: data-parallel over B=8 across the 8 NeuronCores. Each core runs an
identical Bass program over its [F=4, S=512, D=256] slice; host gathers.
"""

import os
import sys
from contextlib import ExitStack

for _p in ("/opt/trn_rl_repo", os.path.expanduser("~/.axon_site/_ro/trn_rl_repo")):
    if os.path.isdir(_p) and _p not in sys.path:
        sys.path.insert(0, _p)

import numpy as np

import concourse.bass as bass
import concourse.tile as tile
from concourse import mybir
from concourse.masks import make_identity

F32 = mybir.dt.float32
F32R = mybir.dt.float32r
AF = mybir.ActivationFunctionType
P = 128

B, F, S, D = 8, 4, 512, 256
H, DK, DV = 8, 32, 32
LN_EPS = 1e-5

ST = S // P          # 4 sequence tiles of 128
KD = D // P          # 2 contraction tiles over D
OT = (H * DK) // P   # 2 tiles over the packed head dim
HPT = 3              # heads per 128-partition tile (matmul operand base
                     # partition must be in {0, 32, 64}, so 3 x 32 rows)
NG = 3               # number of head groups (3 + 3 + 2 heads)
GRP = [(0, 96), (96, 96), (192, 64)]  # (hv col offset, width) per group

# tuning knobs (read at build_program time)
TUNE = dict(res_bufs=4, sc_bufs=7, ps_proj=2, ps_qk=2, ps_ts=2, ps_ctx=2,
            ET_bufs=2, batch2=False, store_ring_act=False, pipe_at=2, defer_st=7, ln_pool=False,
            split_dma=False)



def _emit(ctx: ExitStack, tc: tile.TileContext, nF: int, nH: int, tensors):
    nc = tc.nc
    x_d, res_d, wq_d, wk_d, wv_d, wfc_d, g_d, b_d, scores_d, out_d = tensors

    singles = ctx.enter_context(tc.tile_pool(name="singles", bufs=1))
    perf = ctx.enter_context(tc.tile_pool(name="perf", bufs=2))
    perfh = ctx.enter_context(tc.tile_pool(name="perfh", bufs=TUNE["ET_bufs"]))
    respool = ctx.enter_context(tc.tile_pool(name="respool", bufs=TUNE["res_bufs"]))
    ctxtp = ctx.enter_context(tc.tile_pool(name="ctxtp", bufs=1))
    stg = ctx.enter_context(tc.tile_pool(name="stg", bufs=1))
    scpool = ctx.enter_context(tc.tile_pool(name="scpool", bufs=TUNE["sc_bufs"]))
    ypool = ctx.enter_context(tc.tile_pool(name="ypool", bufs=2))
    # PSUM budget is 8 banks total
    assert TUNE["ps_proj"] + TUNE["ps_qk"] + TUNE["ps_ts"] + TUNE["ps_ctx"] <= 8
    ps_proj = ctx.enter_context(tc.tile_pool(name="ps_proj", bufs=TUNE["ps_proj"], space="PSUM"))
    ps_qk = ctx.enter_context(tc.tile_pool(name="ps_qk", bufs=TUNE["ps_qk"], space="PSUM"))
    ps_ts = ctx.enter_context(tc.tile_pool(name="ps_ts", bufs=TUNE["ps_ts"], space="PSUM"))
    ps_ctx = ctx.enter_context(tc.tile_pool(name="ps_ctx", bufs=TUNE["ps_ctx"], space="PSUM"))

    # load x(0) before the weights: it gates the transpose->projection chain
    x0_sb = perf.tile([P, ST, D], F32, tag="x", name="x0")
    nc.sync.dma_start(out=x0_sb, in_=x_d[0].rearrange("(o p) d -> p o d", p=P))

    ident = singles.tile([P, P], F32)
    make_identity(nc, ident)
    ident_r = singles.tile([P, P], F32R)
    nc.scalar.copy(out=ident_r, in_=ident)

    def load_w(dram, nkt, nm):
        w32 = stg.tile([P, nkt, H * DK], F32, name=nm + "32", tag="wstg")
        nc.sync.dma_start(out=w32, in_=dram.rearrange("(kt p) n -> p kt n", p=P))
        # matmul inputs must be fp32r-rounded; ScalarE rounds on write
        w = singles.tile([P, nkt, H * DK], F32R, name=nm, tag=nm)
        nc.scalar.copy(out=w, in_=w32)
        return w

    wq_sb = load_w(wq_d, KD, "wq_sb")
    wk_sb = load_w(wk_d, KD, "wk_sb")
    wv_sb = load_w(wv_d, KD, "wv_sb")
    wfc32 = stg.tile([DV, H, D], F32, name="wfc32", tag="wstg")
    nc.sync.dma_start(out=wfc32, in_=wfc_d.rearrange("p (h n) -> p h n", h=H))
    wfc_sb = singles.tile([DV, H, D], F32R, name="wfc_sb", tag="wfc_sb")
    nc.scalar.copy(out=wfc_sb, in_=wfc32)

    # ln affine params broadcast across all 128 partitions
    def bcast_vec(dram, nm):
        t = singles.tile([P, D], F32, name=nm, tag=nm)
        src = bass.AP(tensor=dram.tensor, offset=dram.offset,
                      ap=[[0, P]] + list(dram.ap))
        nc.sync.dma_start(out=t, in_=src)
        return t

    g_sb = bcast_vec(g_d[:], "g_sb")
    b_sb = bcast_vec(b_d[:], "b_sb")
    eps_sb = singles.tile([P, 1], F32)
    nc.vector.memset(eps_sb, LN_EPS)
    # dummy Ln pins the natural_log_exp table set (holds Ln AND Exp) so the
    # per-f layernorm below never forces an ACT table switch
    pin_sb = singles.tile([P, 1], F32)
    nc.scalar.activation(out=pin_sb, in_=eps_sb, func=AF.Ln)
    mv_all = singles.tile([P, nF * ST, 2], F32)

    def emit_proj(f, x_pre=None):
        # ---- load x, build xT via PE transpose ----
        if x_pre is not None:
            x_sb = x_pre
        else:
            x_sb = perf.tile([P, ST, D], F32, tag="x", name=f"x{f}")
            nc.sync.dma_start(
                out=x_sb, in_=x_d[f].rearrange("(o p) d -> p o d", p=P))
        xT_sb = perf.tile([P, KD, S], F32R, tag="xT", name=f"xT{f}")
        for dt_ in range(KD):
            pst = ps_proj.tile([P, S], F32, tag="proj", name=f"pstx{f}_{dt_}")
            for st in range(ST):
                nc.tensor.transpose(
                    pst[:, st * P:(st + 1) * P],
                    x_sb[:, st, dt_ * P:(dt_ + 1) * P],
                    ident,
                )
            nc.scalar.copy(out=xT_sb[:, dt_, :], in_=pst)

        # ---- projections: qT/kT in [o, s] layout (3 heads per 128-row
        # tile so head slices sit at legal base partitions); v natural ----
        qT_sb = perf.tile([P, NG, S], F32R, tag="qT", name=f"qT{f}")
        kT_sb = perf.tile([P, NG, S], F32R, tag="kT", name=f"kT{f}")
        for w_sb, dst in ((wq_sb, qT_sb), (wk_sb, kT_sb)):
            for g, (c0, gw) in enumerate(GRP):
                ps = ps_proj.tile([P, S], F32, tag="proj", name=f"psq{f}_{g}")
                for kt in range(KD):
                    nc.tensor.matmul(
                        ps[:gw], w_sb[:, kt, c0:c0 + gw],
                        xT_sb[:, kt, :],
                        start=(kt == 0), stop=(kt == KD - 1),
                    )
                nc.scalar.copy(out=dst[:gw, g, :], in_=ps[:gw])
        v_sb = perf.tile([P, ST, H * DV], F32, tag="v", name=f"v{f}")
        for g2 in range(ST // 2):
            ps = ps_proj.tile([P, 2, H * DV], F32, tag="proj", name=f"psv{f}_{g2}")
            for i2 in range(2):
                tt = g2 * 2 + i2
                for kt in range(KD):
                    nc.tensor.matmul(
                        ps[:, i2, :], xT_sb[:, kt, tt * P:(tt + 1) * P],
                        wv_sb[:, kt, :],
                        start=(kt == 0), stop=(kt == KD - 1),
                    )
            nc.scalar.copy(out=v_sb[:, g2 * 2:g2 * 2 + 2, :], in_=ps)
        return x_sb, qT_sb, kT_sb, v_sb

    cur = emit_proj(0, x_pre=x0_sb)
    deferred = []  # (sc_sb, f, h2) score stores pushed past the compute drain
    for f in range(nF):
        x_sb, qT_sb, kT_sb, v_sb = cur
        nxt = None
        PIPE_AT = TUNE["pipe_at"]

        # ---- attention per head ----
        # per-head ctx^T rows live at partitions 0..32 of one packed tile
        # (4-byte matmuls require destination partition base 0)
        ctxT_sb = ctxtp.tile([DV, H, S], F32R, tag="ctxT")
        HB = 2 if TUNE["batch2"] else 1
        assert nH % HB == 0
        for h2 in range(nH // HB):
            # res load / scores store batched as one DMA per HB heads
            res_sb = respool.tile([P, HB, ST, S], F32, tag="res")
            res_src = res_d[f, HB * h2:HB * h2 + HB].rearrange(
                "i (o p) t -> p i o t", p=P)
            if TUNE["split_dma"]:
                hs = ST // 2
                nc.sync.dma_start(out=res_sb[:, :, :hs, :], in_=res_src[:, :, :hs, :])
                nc.sync.dma_start(out=res_sb[:, :, hs:, :], in_=res_src[:, :, hs:, :])
            else:
                nc.sync.dma_start(out=res_sb, in_=res_src)
            sc_sb = scpool.tile([P, HB, ST, S], F32R, tag="sc")
            for i in range(HB):
                h = HB * h2 + i
                ot_h, hp = h // HPT, (h % HPT) * DK  # group, base partition
                for st in range(ST):
                    ps = ps_qk.tile([P, S], F32, tag="qk")
                    nc.tensor.matmul(
                        ps,
                        qT_sb[hp:hp + DK, ot_h, st * P:(st + 1) * P],
                        kT_sb[hp:hp + DK, ot_h, :],
                        start=True, stop=True,
                    )
                    # fused: scores = qk + res, PSUM -> SBUF
                    i_add = nc.vector.tensor_add(
                        out=sc_sb[:, i, st, :], in0=ps, in1=res_sb[:, i, st, :])
                    if TUNE.get("tail_prio") and f == nF - 1 and h >= nH - 2:
                        i_add.bass_priority = -100
            if f == nF - 1 and h2 >= nH // HB - TUNE["defer_st"]:
                # defer the final stores so DMA stays busy through the
                # end-of-kernel compute drain
                deferred.append((sc_sb, f, h2))
            else:
                st_eng = nc.scalar if TUNE["store_ring_act"] else nc.sync
                st_eng.dma_start(  # stores can ride the ACT HWDGE ring
                    out=scores_d[f, HB * h2:HB * h2 + HB].rearrange(
                        "i (o p) t -> p i o t", p=P),
                    in_=sc_sb.bitcast(F32))
            for i in range(HB):
                h = HB * h2 + i
                # transposed scores -> exp + per-t row sums (softmax over s)
                ET_sb = perfh.tile([P, ST, S], F32R, tag="ET")
                cs_sb = perfh.tile([P, ST], F32, tag="cs")
                for tt in range(ST):
                    ps_t = ps_ts.tile([P, S], F32, tag="ts")
                    for st in range(ST):
                        nc.tensor.transpose(
                            ps_t[:, st * P:(st + 1) * P].bitcast(F32R),
                            sc_sb[:, i, st, tt * P:(tt + 1) * P],
                            ident_r,
                        )
                    nc.scalar.activation(
                        out=ET_sb[:, tt, :], in_=ps_t, func=AF.Exp,
                        accum_out=cs_sb[:, tt:tt + 1])
                rc_sb = perfh.tile([P, ST], F32, tag="rc")
                nc.vector.reciprocal(out=rc_sb, in_=cs_sb)
                vp_sb = perfh.tile([P, ST, DV], F32R, tag="vp")
                for tt in range(ST):
                    nc.vector.tensor_scalar_mul(
                        out=vp_sb[:, tt, :],
                        in0=v_sb[:, tt, h * DV:(h + 1) * DV],
                        scalar1=rc_sb[:, tt:tt + 1])
                ctx_ps = ps_ctx.tile([P, S], F32, tag="ctx")
                for tt in range(ST):
                    nc.tensor.matmul(
                        ctx_ps[:DV, :],
                        vp_sb[:, tt, :], ET_sb[:, tt, :],
                        start=(tt == 0), stop=(tt == ST - 1),
                    )
                nc.vector.tensor_copy(out=ctxT_sb[:, h, :], in_=ctx_ps[:DV, :])
                if h == PIPE_AT and f + 1 < nF:
                    # software-pipeline: emit next frame's projections here so
                    # the scheduler overlaps them with this frame's tail heads
                    nxt = emit_proj(f + 1)

        if nxt is None and f + 1 < nF:
            nxt = emit_proj(f + 1)
        cur = nxt

        # flush deferred stores ahead of the LN/out chain so the SP DMA ring
        # isn't head-of-line blocked behind the out store's LN dependency
        for sc_dv, fv, h2v in deferred:
            nc.sync.dma_start(
                out=scores_d[fv, HB * h2v:HB * h2v + HB].rearrange(
                    "i (o p) t -> p i o t", p=P),
                in_=sc_dv.bitcast(F32))
        deferred.clear()

        # ---- fc + residual + LN stats ----
        y_sb = ypool.tile([P, ST, D], F32, tag="y")
        stats_sb = perf.tile([P, ST, 6], F32, tag="stats")
        for g2 in range(ST // 2):
            ps = ps_proj.tile([P, 2, D], F32, tag="proj")
            for i2 in range(2):
                st = g2 * 2 + i2
                for hh in range(nH):
                    nc.tensor.matmul(
                        ps[:, i2, :],
                        ctxT_sb[:, hh, st * P:(st + 1) * P],
                        wfc_sb[:, hh, :],
                        start=(hh == 0), stop=(hh == nH - 1),
                    )
            for i2 in range(2):
                st = g2 * 2 + i2
                sl = f * ST + st
                nc.vector.tensor_add(
                    out=y_sb[:, st, :], in0=ps[:, i2, :], in1=x_sb[:, st, :])
                nc.vector.bn_stats(out=stats_sb[:, st, :], in_=y_sb[:, st, :])
                nc.vector.bn_aggr(out=mv_all[:, sl, :], in_=stats_sb[:, st, :])
                # per-s-tile layernorm + store keeps the kernel tail short:
                # rstd = exp(-0.5 * ln(var + eps)), same ACT table set as Exp
                lnv = perfh.tile([P, 1], F32, tag="lnv")
                nc.scalar.activation(out=lnv, in_=mv_all[:, sl, 1:2],
                                     func=AF.Ln, bias=eps_sb, scale=1.0)
                rstd = perfh.tile([P, 1], F32, tag="rstd")
                nc.scalar.activation(out=rstd, in_=lnv, func=AF.Exp, scale=-0.5)
                nc.vector.tensor_scalar(
                    out=y_sb[:, st, :], in0=y_sb[:, st, :],
                    scalar1=mv_all[:, sl, 0:1], scalar2=rstd,
                    op0=mybir.AluOpType.subtract, op1=mybir.AluOpType.mult)
                ln_eng = nc.gpsimd if TUNE["ln_pool"] else nc.vector
                ln_eng.tensor_mul(
                    out=y_sb[:, st, :], in0=y_sb[:, st, :], in1=g_sb)
                ln_eng.tensor_add(
                    out=y_sb[:, st, :], in0=y_sb[:, st, :], in1=b_sb)
                nc.sync.dma_start(
                    out=out_d[f].rearrange("(o p) d -> p o d", p=P)[:, st, :],
                    in_=y_sb[:, st, :])


def _split_matmul_waits(nc: bass.Bass) -> None:
    """Hoist extra sem waits onto same-engine EventSemaphore instructions.

    walrus codegen allows only one sync wait per instruction struct (two for
    EventSemaphore); Tile can attach several. Same-engine instructions
    execute in order, so moving all but one wait onto preceding
    EventSemaphores is semantics-preserving.
    """
    for fn in nc.m.functions:
        for blk in fn.blocks:
            insts = list(blk.instructions)
            out = []
            for inst in insts:
                si = inst.sync_info
                splittable = not isinstance(inst, mybir.InstEventSemaphore)
                if (splittable and si is not None
                        and len(si.on_wait) > 1):
                    extra = list(si.on_wait[:-1])
                    # EventSemaphore is the only struct that takes 2 waits;
                    # plain NOPs take none.
                    for j in range(0, len(extra), 2):
                        ev = mybir.InstEventSemaphore(
                            name=nc.get_next_instruction_name(),
                            sync_info=mybir.SyncInfo(
                                on_wait=extra[j:j + 2], on_update=[]),
                            bass_nofuse=True,
                            engine=inst.engine,
                        )
                        out.append(ev)
                    si.on_wait = [si.on_wait[-1]]
                    inst.sync_info = si
                out.append(inst)
            if len(out) != len(insts):
                del blk.instructions[:]
                for i in out:
                    blk.instructions.append(i)


def build_program(nF: int = F, nH: int = H, split_waits: bool = True) -> bass.Bass:
    nc = bass.Bass()
    x_d = nc.declare_dram_parameter("x", [nF, S, D], F32, isOutput=False)
    res_d = nc.declare_dram_parameter("res", [nF, nH, S, S], F32, isOutput=False)
    wq_d = nc.declare_dram_parameter("wq", [D, H * DK], F32, isOutput=False)
    wk_d = nc.declare_dram_parameter("wk", [D, H * DK], F32, isOutput=False)
    wv_d = nc.declare_dram_parameter("wv", [D, H * DV], F32, isOutput=False)
    wfc_d = nc.declare_dram_parameter("wfc", [DV, H * D], F32, isOutput=False)
    g_d = nc.declare_dram_parameter("g", [D], F32, isOutput=False)
    b_d = nc.declare_dram_parameter("b", [D], F32, isOutput=False)
    scores_d = nc.declare_dram_parameter("scores", [nF, nH, S, S], F32, isOutput=True)
    out_d = nc.declare_dram_parameter("out", [nF, S, D], F32, isOutput=True)
    tensors = (x_d, res_d, wq_d, wk_d, wv_d, wfc_d, g_d, b_d, scores_d, out_d)
    with tile.TileContext(nc) as tc:
        with ExitStack() as ctx:
            _emit(ctx, tc, nF, nH, tensors)
    if split_waits:  # CoreSim can't execute the inserted EventSemaphores
        _split_matmul_waits(nc)
    return nc


_CACHE: dict = {}


def _program() -> bass.Bass:
    if "nc" not in _CACHE:
        _CACHE["nc"] = build_program()
    return _CACHE["nc"]


def make_in_maps(x, res_att, W_Q, W_K, W_V, W_fc, ln_g, ln_b):
    f32 = lambda a: np.ascontiguousarray(np.asarray(a), dtype=np.float32)
    x, res_att = f32(x), f32(res_att)
    wq = f32(W_Q) / np.float32(np.sqrt(DK))  # fold 1/sqrt(dk) into W_Q
    wk, wv = f32(W_K), f32(W_V)
    # regroup W_fc rows per head: [H*DV, D] -> [DV, H*D]
    wfc = np.ascontiguousarray(
        f32(W_fc).reshape(H, DV, D).transpose(1, 0, 2).reshape(DV, H * D))
    g, b = f32(ln_g), f32(ln_b)
    return [
        dict(x=x[c], res=res_att[c], wq=wq, wk=wk, wv=wv, wfc=wfc, g=g, b=b)
        for c in range(B)
    ]


def kernel(x, res_att, W_Q, W_K, W_V, W_fc, ln_g, ln_b):
    from concourse.bass_utils import run_bass_kernel_spmd

    in_maps = make_in_maps(x, res_att, W_Q, W_K, W_V, W_fc, ln_g, ln_b)
    r = run_bass_kernel_spmd(_program(), in_maps, list(range(B)))
    out = np.stack([r.results[c]["out"] for c in range(B)])
    scores = np.stack([r.results[c]["scores"] for c in range(B)])
    return out, scores
